# revision 61
# baseline (speedup 1.0000x reference)
"""EntAttentionLayer on 8 TRN2 NeuronCores.

Sharding: pure sequence-parallel, no collectives. Core c handles batch
b = c//4 and query rows [qc*512, qc*512+512), qc = c%4. Each core
computes K/V for its batch's FULL sequence (redundant x4, avoids
collectives), its own 512 queries, and the whole per-row pipeline
(SA -> CA over tags -> FFN) for its rows.

Key device-side tricks:
- bf16 weights + activations on the matmul paths (full PE rate, halved
  DMA); fp32 residuals/psum accumulation throughout.
- Scores computed transposed S^T[k, q] so ctx needs no transpose of E.
- Band mask: keys are ROTATED per-core on the host (softmax is
  permutation-invariant over keys) so the |q-k|<=50 band lands in key
  chunks 0..4 for every core -> uniform SPMD instruction stream; the
  mask itself is per-core input data.
- Softmax denominator: V is augmented with a ones column per head
  (65 cols/head) so each ctx matmul emits [64 ctx rows + 1 denom row].
- Softmax normalize stays on-chip: Pool copies the denom rows out of
  PSUM, DVE runs a batched reciprocal, Pool broadcasts it across the 64
  head partitions, DVE fuses the normalize multiply with the PSUM->SBUF
  move of the context rows.
- The tag table K/V (64 rows) is computed on the host; the CA q/k
  biases are folded into a per-tag scale on the augmented V (exactly,
  via softmax shift-invariance).
- Fused 2-bank exp: both heads of a pair land in adjacent PSUM banks so
  one Activation op covers 1024 columns.
- DVE-only Newton rsqrt for the LayerNorms (no ACT table thrash).
- Attention q/k scale 1/8 folded into Wq on the host.
"""
import sys
sys.path.insert(0, "/opt/trn_rl_repo")
import numpy as np
import ml_dtypes
import concourse.bass as bass
import concourse.mybir as mybir
import concourse.tile as tile
import concourse.bass_isa as bass_isa
from concourse import bacc
from concourse import bass_utils

B, S, D, H, T, RAD = 2, 2048, 768, 12, 64, 50
DH = D // H          # 64
F = 4 * D            # 3072
SQ = S // 4          # 512 query rows per core
P = 128
NC = 8
HA = 65              # aug head width (64 ctx dims + 1 denom)
DA = H * HA          # 780
BAND_COLS = [(0, 114), (14, 242), (142, 370), (270, 498), (398, 512)]
BAND_OFF = [0, 114, 342, 570, 798]
BAND_TOT = 912
F32 = mybir.dt.float32
F32R = mybir.dt.float32r
BF16 = mybir.dt.bfloat16
I32 = mybir.dt.int32
AF = mybir.ActivationFunctionType
ALU = mybir.AluOpType
EPS = 1e-12

_CACHED_NC = None


def _ln_stats(nc, lnp, r_ap, mean4, var4, qt):
    """bn stats of r_ap [P, D]; mean -> mean4[:, qt], var+eps -> var4[:, qt]."""
    st = lnp.tile([P, 3, 6], F32, name="ln_st")
    for g in range(3):
        nc.vector.bn_stats(st[:, g, :], r_ap[:, g * 256:(g + 1) * 256])
    mv = lnp.tile([P, 2], F32, name="ln_mv")
    nc.vector.bn_aggr(mv[:], st[:])
    nc.vector.tensor_copy(mean4[:, qt:qt + 1], mv[:, 0:1])
    nc.vector.tensor_scalar(out=var4[:, qt:qt + 1], in0=mv[:, 1:2],
                            scalar1=EPS, scalar2=None, op0=ALU.add)


def _rsqrt4(nc, lnp, v4, w=4):
    """DVE-only Newton rsqrt of v4 [P, w] (positive). Returns y [P, w]."""
    sh = lnp.tile([P, w], I32, name="rs_sh")
    nc.vector.tensor_scalar(out=sh[:], in0=v4.bitcast(I32), scalar1=1,
                            scalar2=None, op0=ALU.logical_shift_right)
    magic = lnp.tile([P, 1], I32, name="rs_mg")
    nc.vector.memset(magic[:], 0x5F3759DF)
    y = lnp.tile([P, w], F32, name="rs_y")
    nc.vector.tensor_tensor(y[:].bitcast(I32), magic[:].to_broadcast((P, w)),
                            sh[:], ALU.subtract)
    t1 = lnp.tile([P, w], F32, name="rs_t1")
    for _ in range(2):
        nc.vector.tensor_mul(t1[:], v4, y[:])
        nc.vector.tensor_mul(t1[:], t1[:], y[:])
        nc.vector.tensor_scalar(out=t1[:], in0=t1[:], scalar1=-0.5,
                                scalar2=1.5, op0=ALU.mult, op1=ALU.add)
        nc.vector.tensor_mul(y[:], y[:], t1[:])
    return y


def _ln_apply(nc, lnp, r_ap, mean_ap, rs_ap, g_bc, b_bc, out_ap):
    nm = lnp.tile([P, 1], F32, name="ln_nm")
    nc.vector.tensor_scalar(out=nm[:], in0=mean_ap, scalar1=rs_ap,
                            scalar2=-1.0, op0=ALU.mult, op1=ALU.mult)
    t = lnp.tile([P, D], F32, name="ln_t")
    nc.scalar.activation(t[:], r_ap, AF.Identity, bias=nm[:, 0:1],
                         scale=rs_ap)
    nc.vector.tensor_mul(t[:], t[:], g_bc)
    nc.vector.tensor_add(out_ap, t[:], b_bc)


def build_kernel():
    nc = bacc.Bacc("TRN2", target_bir_lowering=False, debug=False,
                   num_devices=NC)

    def din(name, shape, dt=BF16):
        return nc.dram_tensor(name, shape, dt, kind="ExternalInput").ap()

    # --- per-core inputs (weights bf16, residual/bias data f32) ---
    xT = din("xT", [D, S])                        # rotated hidden^T
    xres = din("xres", [SQ, D], F32)              # X rows + sa_bo
    m5 = din("mask5", [P, BAND_TOT], BF16)        # packed band mask (exp values)
    wq = din("wq", [D, D]);  bq = din("bq", [D], F32)      # pre-scaled 1/8
    wk = din("wk", [D, D]);  bk = din("bk", [D], F32)
    wv = din("wv", [D, DA]); bv_bc = din("bv_bc", [P, DA], F32)
    wo = din("wo", [D, D])
    kcaT = din("kcaT", [D, T])                    # host tag K^T
    vca_in = din("vca_in", [T, DA])               # host tag V (aug + bias fold)
    cwq = din("cwq", [D, D])                      # pre-scaled 1/8
    cwo = din("cwo", [D, D]); cbo_bc = din("cbo_bc", [P, D], F32)
    w1 = din("w1", [D, F]); b1p = din("b1p", [P, F // P], F32)
    w2 = din("w2", [F, D]); b2_bc = din("b2_bc", [P, D], F32)
    g1_bc = din("g1_bc", [P, D], F32); b1l_bc = din("b1l_bc", [P, D], F32)
    g2_bc = din("g2_bc", [P, D], F32); b2l_bc = din("b2l_bc", [P, D], F32)
    g3_bc = din("g3_bc", [P, D], F32); b3l_bc = din("b3l_bc", [P, D], F32)
    ident = din("ident", [P, P], F32)
    cbo_r = din("cbo_r", [1, D])                  # CA out bias row (bf16)
    b2_r = din("b2_r", [1, D])                    # FF2 bias row (bf16)
    out = nc.dram_tensor("out", [SQ, D], F32, kind="ExternalOutput").ap()

    # internal DRAM scratch for softmax denominators
    den_dr = nc.dram_tensor("den_dr", [H, SQ], BF16).ap()
    rden_dr = nc.dram_tensor("rden_dr", [H, SQ], F32).ap()

    with tile.TileContext(nc) as tc:
     with tc.tile_pool(name="consts", bufs=1) as consts:
      bq_sb = consts.tile([P, 6], F32, name="bq")
      nc.gpsimd.dma_start(out=bq_sb[:], in_=bq.rearrange("(c p) -> p c", p=P))
      bk_sb = consts.tile([P, 6], F32, name="bk")
      nc.gpsimd.dma_start(out=bk_sb[:], in_=bk.rearrange("(c p) -> p c", p=P))
      ones1 = consts.tile([1, P], BF16, name="ones1")
      nc.vector.memset(ones1[:], 1.0)
      cbor_sb = consts.tile([1, D], BF16, name="cbor")
      nc.gpsimd.dma_start(out=cbor_sb[:], in_=cbo_r)
      b2r_row = consts.tile([1, D], BF16, name="b2rr")
      nc.gpsimd.dma_start(out=b2r_row[:], in_=b2_r)

      with tc.tile_pool(name="zp", bufs=1) as zp:          # z, zT
       with tc.tile_pool(name="att", bufs=1) as att:       # ctxU, tag K/V
        ctxU = att.tile([HA, H, SQ], BF16, name="ctxU")
        kca_sb = att.tile([P, 6, T], BF16, name="kca")
        vca_sb = att.tile([T, DA], BF16, name="vca")
        ident_sb = att.tile([P, P], F32, name="ident")

        if True:
         # helpers shared by SA and CA -------------------------------------
         def ctx_out(hh, cx, on_act=False):
             """ctx PSUM -> ctxU (unnormalized, incl denom row); denom row
             to DRAM for the batched reciprocal."""
             if on_act:
                 nc.scalar.copy(ctxU[:, hh, :], cx[:])
             else:
                 nc.vector.tensor_copy(ctxU[:, hh, :], cx[:])
             nc.sync.dma_start(den_dr[hh:hh + 1, :], ctxU[64:65, hh, :])

         def batch_normalize(heads, dpool, rbpool):
             """One reciprocal for len(heads) denominators, then
             broadcast-DMA + in-place normalize muls on ctxU."""
             n = len(heads)
             h0 = heads[0]
             dhb = dpool.tile([n, SQ], BF16, name="dhb", tag="dhb")
             nc.sync.dma_start(dhb[:], den_dr[h0:h0 + n, :])
             dh = dpool.tile([n, SQ], F32, name="dh", tag="dh")
             nc.vector.tensor_copy(dh[:], dhb[:])
             rdh = dpool.tile([n, SQ], F32, name="rdh", tag="rdh")
             scr = dpool.tile([n, SQ], F32, name="scr", tag="scr")
             nc.vector.reciprocal_approx_accurate(rdh[:], dh[:], scr[:])
             nc.sync.dma_start(rden_dr[h0:h0 + n, :], rdh[:])
             for hh in heads:
                 rb = rbpool.tile([64, SQ], F32, name="rb2")
                 nc.gpsimd.dma_start(
                     out=rb[:],
                     in_=rden_dr[hh:hh + 1, :].to_broadcast((64, SQ)))
                 nc.vector.tensor_mul(ctxU[0:64, hh, :],
                                      ctxU[0:64, hh, :], rb[:])

         with tc.tile_pool(name="pf", bufs=1) as pf:       # stage2-4a operands
            # ---------- Stage 2: self-attention ----------
            HH = DA // 2  # 390 aug cols per half
            with tc.tile_pool(name="xt", bufs=1) as xtp, \
                 tc.tile_pool(name="m5p", bufs=1) as m5p, \
                 tc.tile_pool(name="kv", bufs=1) as kvp, \
                 tc.tile_pool(name="wst", bufs=1) as wst, \
                 tc.tile_pool(name="ep", bufs=3) as epool, \
                 tc.tile_pool(name="dnp", bufs=2) as dnp, \
                 tc.tile_pool(name="rbp2", bufs=2) as rbp2:
                # ---- DMA issue order controls the critical path ----
                xT_sb = xtp.tile([P, 6, S], BF16, name="xT")
                xTr = xT.rearrange("(c p) s -> p c s", p=P)
                wv_ts = [wst.tile([P, 6, HH], BF16, name="wv_t", bufs=2)
                         for _ in range(2)]
                nc.sync.dma_start(
                    wv_ts[0][:],
                    wv.rearrange("(c p) e -> p c e", p=P)[:, :, 0:HH])
                nc.sync.dma_start(xT_sb[:, :, 0:512], xTr[:, :, 0:512])
                wkq = {}
                for half in (0, 1):
                    wkq[("k", half)] = wst.tile([P, 6, 3 * P], BF16,
                                                name="wkq_t", tag="wkq",
                                                bufs=2)
                    wkq[("q", half)] = wst.tile([P, 6, 3 * P], BF16,
                                                name="wkq_t", tag="wkq",
                                                bufs=2)
                nc.sync.dma_start(
                    wkq[("k", 0)][:],
                    wk.rearrange("(c p) e -> p c e", p=P)[:, :, 0:384])
                nc.sync.dma_start(
                    wkq[("q", 0)][:],
                    wq.rearrange("(c p) e -> p c e", p=P)[:, :, 0:384])
                bv_sb = xtp.tile([P, DA], F32, name="bv")
                nc.sync.dma_start(bv_sb[:], bv_bc)
                m5_sb = m5p.tile([P, BAND_TOT], BF16, name="m5")
                nc.sync.dma_start(m5_sb[:], m5)
                for scc in range(1, 4):
                    nc.sync.dma_start(
                        xT_sb[:, :, scc * 512:(scc + 1) * 512],
                        xTr[:, :, scc * 512:(scc + 1) * 512])
                nc.sync.dma_start(
                    wv_ts[1][:],
                    wv.rearrange("(c p) e -> p c e", p=P)[:, :, HH:2 * HH])
                nc.sync.dma_start(
                    wkq[("k", 1)][:],
                    wk.rearrange("(c p) e -> p c e", p=P)[:, :, 384:768])
                nc.sync.dma_start(
                    wkq[("q", 1)][:],
                    wq.rearrange("(c p) e -> p c e", p=P)[:, :, 384:768])
                # stage-3/4 prefetches (DMA engines idle during SA)
                wo_t = pf.tile([64, H, D], BF16, name="wo_t")
                nc.sync.dma_start(wo_t[:],
                                  wo.rearrange("(h p) e -> p h e", p=64))
                xres_sb = pf.tile([P, 4, D], F32, name="xres")
                nc.sync.dma_start(xres_sb[:],
                                  xres.rearrange("(q p) e -> p q e", p=P))
                cwq_t = pf.tile([P, 6, D], BF16, name="cwq_t")
                nc.sync.dma_start(cwq_t[:],
                                  cwq.rearrange("(c p) e -> p c e", p=P))
                g1_sb = pf.tile([P, D], F32, name="g1")
                nc.sync.dma_start(g1_sb[:], g1_bc)
                b1l_sb = pf.tile([P, D], F32, name="b1l")
                nc.sync.dma_start(b1l_sb[:], b1l_bc)
                nc.sync.dma_start(ident_sb[:], ident)
                nc.sync.dma_start(kca_sb[:],
                                  kcaT.rearrange("(c p) t -> p c t", p=P))
                nc.sync.dma_start(vca_sb[:], vca_in)

                def v_proj(half, pj):
                    wv_t = wv_ts[half]
                    v_sb = kvp.tile([P, 16, HH], BF16, name="v", bufs=2)
                    for sc in range(16):
                        ps = pj.tile([P, 512], F32, name="ps_pj")
                        for cc in range(6):
                            nc.tensor.matmul(
                                ps[:, 0:HH], xT_sb[:, cc, sc * P:(sc + 1) * P],
                                wv_t[:, cc, :],
                                start=(cc == 0), stop=(cc == 5))
                        nc.vector.tensor_add(
                            v_sb[:, sc, :], ps[:, 0:HH],
                            bv_sb[:, half * HH:(half + 1) * HH])
                    return v_sb

                def kq_proj(half, pj):
                    kT_sb = kvp.tile([P, 3, S], BF16, name="kT", bufs=2)
                    qT_sb = kvp.tile([P, 3, SQ], BF16, name="qT", bufs=2)
                    wk_t, wq_t = wkq[("k", half)], wkq[("q", half)]
                    for dcl in range(3):
                        dc = half * 3 + dcl
                        for scc in range(4):
                            ps = pj.tile([P, 512], F32, name="ps_pj")
                            for cc in range(6):
                                nc.tensor.matmul(
                                    ps[:], wk_t[:, cc, dcl * P:(dcl + 1) * P],
                                    xT_sb[:, cc, scc * 512:(scc + 1) * 512],
                                    start=(cc == 0), stop=(cc == 5))
                            nc.vector.tensor_scalar(
                                out=kT_sb[:, dcl, scc * 512:(scc + 1) * 512],
                                in0=ps[:], scalar1=bk_sb[:, dc:dc + 1],
                                scalar2=None, op0=ALU.add)
                    for dcl in range(3):
                        dc = half * 3 + dcl
                        ps = pj.tile([P, 512], F32, name="ps_pj")
                        for cc in range(6):
                            nc.tensor.matmul(ps[:],
                                             wq_t[:, cc, dcl * P:(dcl + 1) * P],
                                             xT_sb[:, cc, 64:64 + SQ],
                                             start=(cc == 0), stop=(cc == 5))
                        nc.vector.tensor_scalar(out=qT_sb[:, dcl, :], in0=ps[:],
                                                scalar1=bq_sb[:, dc:dc + 1],
                                                scalar2=None, op0=ALU.add)
                    return kT_sb, qT_sb

                def sa_pairs(half, kT_sb, qT_sb, v_sb, norm_plan=None):
                    with tc.tile_pool(name="scs", bufs=2, space="PSUM") as scs, \
                         tc.tile_pool(name="cxs", bufs=2, space="PSUM") as cxs:
                        for pl in range(3):
                            pg = half * 3 + pl
                            ha, hb = 2 * pg, 2 * pg + 1
                            la, lb = 2 * pl, 2 * pl + 1
                            ctxA = cxs.tile([HA, SQ], F32, name="ctx")
                            ctxB = cxs.tile([HA, SQ], F32, name="ctx")
                            for kc in range(16):
                                s2 = scs.tile([P, 2, SQ], F32, name="s")
                                nc.tensor.matmul(
                                    s2[:, 0, :],
                                    kT_sb[0:64, pl, kc * P:(kc + 1) * P],
                                    qT_sb[0:64, pl, :], start=True, stop=True)
                                nc.tensor.matmul(
                                    s2[:, 1, :],
                                    kT_sb[64:P, pl, kc * P:(kc + 1) * P],
                                    qT_sb[64:P, pl, :], start=True, stop=True)
                                e2 = epool.tile([P, 2, SQ], BF16, name="e")
                                nc.scalar.activation(e2[:], s2[:], AF.Exp)
                                if kc < 5:
                                    lo, hi = BAND_COLS[kc]
                                    mo = BAND_OFF[kc]
                                    for j in (0, 1):
                                        nc.vector.tensor_tensor(
                                            e2[:, j, lo:hi], e2[:, j, lo:hi],
                                            m5_sb[:, mo:mo + hi - lo], ALU.mult)
                                nc.tensor.matmul(
                                    ctxA[:], v_sb[:, kc, la * HA:(la + 1) * HA],
                                    e2[:, 0, :], start=(kc == 0), stop=(kc == 15))
                                nc.tensor.matmul(
                                    ctxB[:], v_sb[:, kc, lb * HA:(lb + 1) * HA],
                                    e2[:, 1, :], start=(kc == 0), stop=(kc == 15))
                            ctx_out(ha, ctxA)
                            ctx_out(hb, ctxB)
                            if norm_plan and pl in norm_plan:
                                batch_normalize(norm_plan[pl], dnp, rbp2)

                with tc.tile_pool(name="pj", bufs=2, space="PSUM") as pj:
                    v0 = v_proj(0, pj)
                    k0, q0 = kq_proj(0, pj)
                    sa_pairs(0, k0, q0, v0,
                             norm_plan={1: [0, 1, 2, 3], 2: [4, 5]})
                    v1 = v_proj(1, pj)
                    k1, q1 = kq_proj(1, pj)
                # po1's PSUM pool opens BEFORE sa1 so its banks don't
                # alias sa1's score/ctx banks: pass-1 out-proj matmuls can
                # then fill sa1's Act-bound bubbles instead of waiting on a
                # bank-reuse WAR.
                with tc.tile_pool(name="po1", bufs=1, space="PSUM") as po1:
                    sa_pairs(1, k1, q1, v1,
                             norm_plan={1: [6, 7, 8, 9], 2: [10, 11]})

                    # SA out-proj pass 1 (heads 0-5) in-place into xres
                    for qt in range(4):
                        po = po1.tile([P, D], F32, name="po")
                        for h in range(6):
                            nc.tensor.matmul(
                                po[:, 0:512],
                                ctxU[0:64, h, qt * P:(qt + 1) * P],
                                wo_t[:, h, 0:512],
                                start=(h == 0), stop=(h == 5))
                            nc.tensor.matmul(
                                po[:, 512:D],
                                ctxU[0:64, h, qt * P:(qt + 1) * P],
                                wo_t[:, h, 512:D],
                                start=(h == 0), stop=(h == 5))
                        nc.vector.tensor_add(xres_sb[:, qt, :],
                                             xres_sb[:, qt, :], po[:])

            # ---------- Stage 3: out-proj pass 2, LN1, A^T ----------
            p34 = tc.alloc_tile_pool(name="p34", bufs=1,
                                     side="right")
            a_sb = p34.tile([P, 4, D], F32, name="a_sb")
            aT_sb = p34.tile([P, 6, SQ], BF16, name="aT")
            qcaT_sb = p34.tile([P, 6, SQ], BF16, name="qcaT")
            cwo_t = p34.tile([64, H, D], BF16, name="cwo_t")
            cwor = cwo.rearrange("(h p) e -> p h e", p=64)
            nc.scalar.dma_start(cwo_t[:, 0:6, :], cwor[:, 0:6, :])
            nc.scalar.dma_start(cwo_t[:, 6:H, :], cwor[:, 6:H, :])

            ffp = tc.alloc_tile_pool(name="ffp", bufs=1,
                                     side="right")
            ig_sb = ffp.tile([P, F // P, SQ], BF16, name="ig")
            b1p_sb = ffp.tile([P, F // P, 1], F32, name="b1p")
            w1tiles = [ffp.tile([P, 6, F // 6], BF16, name="w1_r", tag="w1r",
                                bufs=6) for _ in range(6)]
            with tc.tile_pool(name="st3", bufs=1) as st3, \
                 tc.tile_pool(name="lnp", bufs=3) as lnp, \
                 tc.tile_pool(name="pso", bufs=3, space="PSUM") as pso, \
                 tc.tile_pool(name="pst", bufs=2, space="PSUM") as pst:
                mean4 = st3.tile([P, 4], F32, name="mean4")
                var4 = st3.tile([P, 4], F32, name="var4")
                for qt in range(4):
                    po = pso.tile([P, D], F32, name="po2")
                    for hl in range(6):
                        h = 6 + hl
                        nc.tensor.matmul(
                            po[:, 0:512],
                            ctxU[0:64, h, qt * P:(qt + 1) * P],
                            wo_t[:, h, 0:512],
                            start=(hl == 0), stop=(hl == 5))
                        nc.tensor.matmul(
                            po[:, 512:D],
                            ctxU[0:64, h, qt * P:(qt + 1) * P],
                            wo_t[:, h, 512:D],
                            start=(hl == 0), stop=(hl == 5))
                    r = st3.tile([P, D], F32, name=f"r{qt}")
                    nc.vector.tensor_add(r[:], xres_sb[:, qt, :], po[:])
                    _ln_stats(nc, lnp, r[:], mean4, var4, qt)
                    rs1 = _rsqrt4(nc, lnp, var4[:, qt:qt + 1], w=1)
                    _ln_apply(nc, lnp, r[:], mean4[:, qt:qt + 1],
                              rs1[:, 0:1],
                              g1_sb[:], b1l_sb[:], a_sb[:, qt, :])
                    for ec in range(6):
                        pt = pst.tile([P, P], F32, name="pt")
                        nc.tensor.transpose(
                            pt[:], a_sb[:, qt, ec * P:(ec + 1) * P],
                            ident_sb[:])
                        nc.scalar.copy(
                            aT_sb[:, ec, qt * P:(qt + 1) * P], pt[:])

            # ---------- Stage 4a: CA q-projection (needs cwq_t from pf) ----
            with tc.tile_pool(name="ps4", bufs=2, space="PSUM") as ps4:
                for dc in range(6):
                    ps = ps4.tile([P, 512], F32, name="ps4t")
                    for cc in range(6):
                        nc.tensor.matmul(
                            ps[:], cwq_t[:, cc, dc * P:(dc + 1) * P],
                            aT_sb[:, cc, :],
                            start=(cc == 0), stop=(cc == 5))
                    nc.scalar.copy(qcaT_sb[:, dc, :], ps[:])

            # FF1 weights + gelu bias: issue on the Act queue now, ahead of
            # the CA work, so they stream during the CA latency window.
            # Only the first two chunks here (2 rotating slots); the rest
            # are emitted inside the FF1 loop as slots free up.
            w1re = w1.rearrange("(c p) e -> p c e", p=P)
            nc.scalar.dma_start(b1p_sb[:], b1p[:, :, None])
            HF6 = F // 12
            for q6 in range(6):
                for hf in range(2):
                    nc.scalar.dma_start(
                        w1tiles[q6][:, :, hf * HF6:(hf + 1) * HF6],
                        w1re[:, :, q6 * (F // 6) + hf * HF6:
                             q6 * (F // 6) + (hf + 1) * HF6])
         # pf closes here: wo/xres/cwq/g1/b1l space freed for stage 5

         # ---------- Stage 4b: cross-attention, LN2, Z^T ----------
         with tc.tile_pool(name="st4", bufs=1) as st4, \
              tc.tile_pool(name="lnp4", bufs=3) as lnp4, \
              tc.tile_pool(name="e4p", bufs=3) as e4p, \
              tc.tile_pool(name="dnp4", bufs=2) as dnp4, \
              tc.tile_pool(name="rbp4", bufs=2) as rbp4:
            g2_sb = st4.tile([P, D], F32, name="g2")
            nc.scalar.dma_start(g2_sb[:], g2_bc)
            b2l_sb = st4.tile([P, D], F32, name="b2l")
            nc.scalar.dma_start(b2l_sb[:], b2l_bc)

            with tc.tile_pool(name="sc4", bufs=2, space="PSUM") as sc4, \
                 tc.tile_pool(name="cx4", bufs=2, space="PSUM") as cx4:
                for pg in range(6):
                    ha, hb = 2 * pg, 2 * pg + 1
                    s2 = sc4.tile([T, 2, SQ], F32, name="s4")
                    nc.tensor.matmul(s2[:, 0, :], kca_sb[0:64, pg, :],
                                     qcaT_sb[0:64, pg, :],
                                     start=True, stop=True)
                    nc.tensor.matmul(s2[:, 1, :], kca_sb[64:P, pg, :],
                                     qcaT_sb[64:P, pg, :],
                                     start=True, stop=True)
                    e2 = e4p.tile([T, 2, SQ], BF16, name="e4")
                    nc.scalar.activation(e2[:], s2[:], AF.Exp)
                    cxA = cx4.tile([HA, SQ], F32, name="cx4t")
                    cxB = cx4.tile([HA, SQ], F32, name="cx4t")
                    for j, cx, hh in ((0, cxA, ha), (1, cxB, hb)):
                        nc.tensor.matmul(
                            cx[:],
                            vca_sb[:, hh * HA:(hh + 1) * HA],
                            e2[:, j, :], start=True, stop=True)
                    ctx_out(ha, cxA, on_act=True)
                    ctx_out(hb, cxB, on_act=False)
                    if pg in (1, 3, 5):
                        batch_normalize([2 * pg - 2, 2 * pg - 1,
                                         2 * pg, 2 * pg + 1], dnp4, rbp4)

            z_sb = zp.tile([P, 4, D], F32, name="z_sb")
            zT_sb = zp.tile([P, 6, SQ], BF16, name="zTs")
            with tc.tile_pool(name="pso4", bufs=2, space="PSUM") as pso4, \
                 tc.tile_pool(name="pst4", bufs=2, space="PSUM") as pst4:
                mean4 = st4.tile([P, 4], F32, name="mean4")
                var4 = st4.tile([P, 4], F32, name="var4")
                for qt in range(4):
                    po = pso4.tile([P, D], F32, name="po4")
                    for h in range(H):
                        nc.tensor.matmul(
                            po[:, 0:512],
                            ctxU[0:64, h, qt * P:(qt + 1) * P],
                            cwo_t[:, h, 0:512],
                            start=(h == 0), stop=False)
                        nc.tensor.matmul(
                            po[:, 512:D],
                            ctxU[0:64, h, qt * P:(qt + 1) * P],
                            cwo_t[:, h, 512:D],
                            start=(h == 0), stop=False)
                    nc.tensor.matmul(po[:, 0:512], ones1[0:1, 0:P],
                                     cbor_sb[0:1, 0:512],
                                     start=False, stop=True)
                    nc.tensor.matmul(po[:, 512:D], ones1[0:1, 0:P],
                                     cbor_sb[0:1, 512:D],
                                     start=False, stop=True)
                    r = st4.tile([P, D], F32, name=f"r4{qt}")
                    nc.vector.tensor_add(r[:], a_sb[:, qt, :], po[:])
                    _ln_stats(nc, lnp4, r[:], mean4, var4, qt)
                    rs1 = _rsqrt4(nc, lnp4, var4[:, qt:qt + 1], w=1)
                    _ln_apply(nc, lnp4, r[:], mean4[:, qt:qt + 1],
                              rs1[:, 0:1],
                              g2_sb[:], b2l_sb[:], z_sb[:, qt, :])
                    for ec in range(6):
                        pt = pst4.tile([P, P], F32, name="pt4")
                        nc.tensor.transpose(
                            pt[:], z_sb[:, qt, ec * P:(ec + 1) * P],
                            ident_sb[:])
                        nc.scalar.copy(
                            zT_sb[:, ec, qt * P:(qt + 1) * P], pt[:])

         # ---------- Stage 5: FFN + LN3 + output ----------
         with tc.tile_pool(name="st5", bufs=1) as st5, \
              tc.tile_pool(name="lnp5", bufs=3) as lnp5:
            w2_sb = st5.tile([P, F // P, D], BF16, name="w2_sb")
            w2r2 = w2.rearrange("(c p) e -> p c e", p=P)
            for q4 in range(6):
                nc.sync.dma_start(w2_sb[:, q4 * 4:(q4 + 1) * 4, :],
                                  w2r2[:, q4 * 4:(q4 + 1) * 4, :])
            g3_sb = st5.tile([P, D], F32, name="g3")
            nc.sync.dma_start(g3_sb[:], g3_bc)
            b3l_sb = st5.tile([P, D], F32, name="b3l")
            nc.sync.dma_start(b3l_sb[:], b3l_bc)
            mean4 = st5.tile([P, 4], F32, name="mean4")
            var4 = st5.tile([P, 4], F32, name="var4")
            # FF1: q6-outer, w1 chunks roll through 2 slots; the zT column
            # halves let FF1 start before the last LN2 q-tiles finish.
            with tc.tile_pool(name="ps5", bufs=4, space="PSUM") as ps5:
                for qh in range(2):
                    for q6 in range(6):
                        w1_t = w1tiles[q6]
                        for i in range(4):
                            fc = q6 * 4 + i
                            ps = ps5.tile([P, 256], F32, name="ps5t")
                            for cc in range(6):
                                nc.tensor.matmul(
                                    ps[:],
                                    w1_t[:, cc, i * P:(i + 1) * P],
                                    zT_sb[:, cc, qh * 256:(qh + 1) * 256],
                                    start=(cc == 0), stop=(cc == 5))
                            nc.scalar.activation(
                                ig_sb[:, fc, qh * 256:(qh + 1) * 256],
                                ps[:], AF.Gelu, bias=b1p_sb[:, fc, 0:1])
            # FF2: per q-tile from SBUF-resident w2; LN3 of qt overlaps qt+1
            with tc.tile_pool(name="pso5", bufs=2, space="PSUM") as pso5:
                for qt in range(4):
                    po = pso5.tile([P, D], F32, name="po5")
                    for fc in range(F // P):
                        nc.tensor.matmul(po[:, 0:512],
                                         ig_sb[:, fc, qt * P:(qt + 1) * P],
                                         w2_sb[:, fc, 0:512],
                                         start=(fc == 0), stop=False)
                        nc.tensor.matmul(po[:, 512:D],
                                         ig_sb[:, fc, qt * P:(qt + 1) * P],
                                         w2_sb[:, fc, 512:D],
                                         start=(fc == 0), stop=False)
                    nc.tensor.matmul(po[:, 0:512], ones1[0:1, 0:P],
                                     b2r_row[0:1, 0:512],
                                     start=False, stop=True)
                    nc.tensor.matmul(po[:, 512:D], ones1[0:1, 0:P],
                                     b2r_row[0:1, 512:D],
                                     start=False, stop=True)
                    r = st5.tile([P, D], F32, name="r5", tag="r5", bufs=2)
                    nc.vector.tensor_add(r[:], z_sb[:, qt, :], po[:])
                    _ln_stats(nc, lnp5, r[:], mean4, var4, qt)
                    rs1 = _rsqrt4(nc, lnp5, var4[:, qt:qt + 1], w=1)
                    o_sb = lnp5.tile([P, D], F32, name="o5")
                    _ln_apply(nc, lnp5, r[:], mean4[:, qt:qt + 1],
                              rs1[:, 0:1],
                              g3_sb[:], b3l_sb[:], o_sb[:])
                    nc.sync.dma_start(out[qt * P:(qt + 1) * P, :],
                                      o_sb[:])
         ffp.release()
         p34.release()

    nc.compile()
    return nc


def _prep_shared(inp):
    """Host-side shared (core-independent) arrays."""
    f32 = np.float32
    bf16 = ml_dtypes.bfloat16

    def bfc(a):
        return np.ascontiguousarray(np.asarray(a, f32).astype(bf16))

    sh = {}
    sh["wq"] = bfc(inp["sa_wq"] * 0.125)
    sh["bq"] = np.ascontiguousarray(inp["sa_bq"] * 0.125)
    sh["wk"] = bfc(inp["sa_wk"])
    sh["bk"] = np.ascontiguousarray(inp["sa_bk"])

    def aug(wv, bv):
        wva = np.zeros((D, DA), f32)
        bva = np.zeros((DA,), f32)
        for h in range(H):
            wva[:, h * HA:h * HA + DH] = wv[:, h * DH:(h + 1) * DH]
            bva[h * HA:h * HA + DH] = bv[h * DH:(h + 1) * DH]
            bva[h * HA + DH] = 1.0
        return wva, bva

    wva, bva = aug(inp["sa_wv"], inp["sa_bv"])
    sh["wv"] = bfc(wva)
    sh["bv_bc"] = np.ascontiguousarray(np.broadcast_to(bva, (P, DA)))
    sh["wo"] = bfc(inp["sa_wo"])

    # ---- host tag-table K/V for the cross-attention ----
    # K_t = tag_emb @ cwk + cbk ; q_scaled = a @ (cwq/8) on device.
    # Softmax shift-invariance over tags folds the cbq-dependent score
    # term exp((cbq/8) . K_t) into a per-tag scale on the augmented V;
    # the per-query term exp(q . cbk-ish) cancels in the normalize.
    ca_k = np.asarray(inp["tag_emb"] @ inp["ca_wk"] + inp["ca_bk"], f32)
    cbq_s = np.asarray(inp["ca_bq"], f32) * 0.125
    kh = ca_k.reshape(T, H, DH)                                  # [T, H, DH]
    s0 = np.einsum("thd,hd->ht", kh, cbq_s.reshape(H, DH))       # [H, T]
    ca_v = np.asarray(inp["tag_emb"] @ inp["ca_wv"] + inp["ca_bv"], f32)
    vca = np.zeros((T, DA), f32)
    for h in range(H):
        fac = np.exp(s0[h])[:, None]                              # [T, 1]
        vca[:, h * HA:h * HA + DH] = ca_v[:, h * DH:(h + 1) * DH] * fac
        vca[:, h * HA + DH] = fac[:, 0]
    sh["vca_in"] = bfc(vca)
    sh["kcaT"] = bfc(ca_k.T)

    sh["cwq"] = bfc(inp["ca_wq"] * 0.125)
    sh["cwo"] = bfc(inp["ca_wo"])
    sh["cbo_bc"] = np.ascontiguousarray(np.broadcast_to(inp["ca_bo"], (P, D)))
    sh["w1"] = bfc(inp["ff_w1"])
    sh["b1p"] = np.ascontiguousarray(
        np.asarray(inp["ff_b1"], f32).reshape(F // P, P).T)
    sh["w2"] = bfc(inp["ff_w2"])
    sh["b2_bc"] = np.ascontiguousarray(np.broadcast_to(inp["ff_b2"], (P, D)))
    sh["cbo_r"] = bfc(np.asarray(inp["ca_bo"], f32).reshape(1, D))
    sh["b2_r"] = bfc(np.asarray(inp["ff_b2"], f32).reshape(1, D))
    sh["g1_bc"] = np.ascontiguousarray(np.broadcast_to(inp["sa_ln_g"], (P, D)))
    sh["b1l_bc"] = np.ascontiguousarray(np.broadcast_to(inp["sa_ln_b"], (P, D)))
    sh["g2_bc"] = np.ascontiguousarray(np.broadcast_to(inp["ca_ln_g"], (P, D)))
    sh["b2l_bc"] = np.ascontiguousarray(np.broadcast_to(inp["ca_ln_b"], (P, D)))
    sh["g3_bc"] = np.ascontiguousarray(np.broadcast_to(inp["ff_ln_g"], (P, D)))
    sh["b3l_bc"] = np.ascontiguousarray(np.broadcast_to(inp["ff_ln_b"], (P, D)))
    sh["ident"] = np.eye(P, dtype=f32)
    return sh


def _mask5_for(qc):
    q0 = qc * SQ
    pos = np.arange(5 * P)
    s_true = (pos - 64 + q0) % S
    u = np.arange(SQ)
    band = (np.abs((q0 + u)[None, :] - s_true[:, None]) <= RAD)
    bexp = np.where(band, np.float32(np.e), np.float32(1.0)).astype(np.float32)
    bexp = bexp.reshape(5, P, SQ).transpose(1, 0, 2)  # [P, 5, SQ]
    packed = np.empty((P, BAND_TOT), ml_dtypes.bfloat16)
    for j, (lo, hi) in enumerate(BAND_COLS):
        packed[:, BAND_OFF[j]:BAND_OFF[j] + hi - lo] = bexp[:, j, lo:hi]
    return np.ascontiguousarray(packed)


def _make_in_maps(inp):
    sh = _prep_shared(inp)
    masks = [_mask5_for(qc) for qc in range(4)]
    hs = inp["hidden_states"]
    in_maps = []
    for c in range(NC):
        b, qc = c // 4, c % 4
        q0 = qc * SQ
        xTb = np.ascontiguousarray(hs[b].T)
        m = dict(sh)
        m["xT"] = np.ascontiguousarray(
            np.roll(xTb, 64 - q0, axis=1).astype(ml_dtypes.bfloat16))
        m["xres"] = np.ascontiguousarray(hs[b, q0:q0 + SQ] + inp["sa_bo"])
        m["mask5"] = masks[qc]
        in_maps.append(m)
    return in_maps


def kernel(**inputs):
    global _CACHED_NC
    inp = {k: np.asarray(v, dtype=np.float32) for k, v in inputs.items()}
    if _CACHED_NC is None:
        _CACHED_NC = build_kernel()
    nc = _CACHED_NC

    in_maps = _make_in_maps(inp)
    res = bass_utils.run_bass_kernel_spmd(nc, in_maps, core_ids=list(range(NC)))
    out = np.empty((B, S, D), np.float32)
    for c in range(NC):
        b, qc = c // 4, c % 4
        out[b, qc * SQ:(qc + 1) * SQ] = res.results[c]["out"]
    return out
